# revision 1
# baseline (speedup 1.0000x reference)
"""DispEstimator Trainium2 kernel: 8-core SPMD (batch x H-half sharding).

Core c handles sample b=c//2, vertical half h=c%2. Odd cores get a vertically
flipped view of their sample (dy-flipped weights, permuted lc-channel order
for e1), so every core runs identical code: the "top" slab edge is a true
image boundary, the "bottom" edge is interior halo. Instance-norm and
correlation-normalization statistics are made global via pairwise AllReduce
between the two cores of each sample.

Correlation: lc_k = (A + B_shift - 2*C_k)/64 with A = sum_c f2^2,
B = sum_c f1s^2, C_k = sum_c f2 * shift_k(f1s). C is computed on
DVE (products) + TensorE (pairwise block-ones reduction). A rides as an extra
e1 input channel (summed weights); B's contribution is folded into a 9x9
single-channel conv on B executed as extra accumulating matmuls in e1. The
1/(64*s) normalization is baked into the e1/B weight tiles after AllReduce.
"""
import sys

import numpy as np

if "/opt/trn_rl_repo" not in sys.path:
    sys.path.insert(0, "/opt/trn_rl_repo")

import bass_rust
import concourse.bass as bass
import concourse.mybir as mybir
from concourse.bass_utils import run_bass_kernel_spmd
from concourse.tile import TileContext

F32 = mybir.dt.float32
F32R = mybir.dt.float32r
AF = mybir.ActivationFunctionType
ALU = mybir.AluOpType
AX = mybir.AxisListType

B, CH, H, W = 4, 64, 256, 256
HALF = 128
HALO = 18
SLAB = HALO + HALF + HALO  # 164
WP = W + 8                 # 264, data at cols X0..X0+255
X0 = 4
R0 = HALO                  # slab row of image row 0

EPS_CORR = 1e-6
EPS_IN = 1e-5

ROWS_F1 = (18, 162)
ROWS_T1 = (18, 161)
ROWS_FEAT = (18, 160)
ROWS_F1S = (18, 159)
ROWS_CORR = (18, 156)
ROWS_E1 = (18, 155)
ROWS_E2 = (18, 153)
ROWS_E3 = (18, 149)
ROWS_HEAD = (18, 148)
STAT_LO, STAT_HI = 18, 145

GROUPS = [[0, 1], [2, 3], [4, 5], [6, 7]]
TAPS = [(dy - 1, dx - 1) for dy in range(3) for dx in range(3)]


def _gauss1d():
    x = np.arange(7, dtype=np.float32) - 3.0
    g = np.exp(-(x ** 2) / (2.0 * 1.5 ** 2))
    return (g / g.sum()).astype(np.float32)


GG = _gauss1d()


# ---------------------------------------------------------------- host prep
def _tapT(w):
    """[O, I, 3, 3] -> [9, I, O] tap-major stationary layout."""
    o, i, _, _ = w.shape
    return np.ascontiguousarray(w.transpose(2, 3, 1, 0).reshape(9, i, o))


def _disp_vmat(rin_lo, rin_hi, ro_lo, ro_hi, clamp_lo):
    nin = rin_hi - rin_lo + 1
    nout = ro_hi - ro_lo + 1
    m = np.zeros((2 * nin, 2 * nout), np.float32)
    for ro in range(ro_lo, ro_hi + 1):
        for d in range(7):
            ri = ro + d - 3
            if clamp_lo is not None:
                ri = max(ri, clamp_lo)
            assert rin_lo <= ri <= rin_hi, (ro, d, ri)
            for c in range(2):
                m[2 * (ri - rin_lo) + c, 2 * (ro - ro_lo) + c] += GG[d]
    return m


def _prep_core_inputs(inp, core):
    b, h = core // 2, core % 2
    f1 = np.asarray(inp["feat1"][b], np.float32)
    f2 = np.asarray(inp["feat2"][b], np.float32)
    pre_w = np.asarray(inp["pre_w"]); pre_b = np.asarray(inp["pre_b"])
    fc1_w = np.asarray(inp["fc1_w"]); fc1_g = np.asarray(inp["fc1_g"]); fc1_be = np.asarray(inp["fc1_be"])
    fc2_w = np.asarray(inp["fc2_w"]); fc2_b = np.asarray(inp["fc2_b"])
    e1_w = np.asarray(inp["e1_w"]); e1_g = np.asarray(inp["e1_g"]); e1_be = np.asarray(inp["e1_be"])
    e2_w = np.asarray(inp["e2_w"]); e2_g = np.asarray(inp["e2_g"]); e2_be = np.asarray(inp["e2_be"])
    e3_w = np.asarray(inp["e3_w"]); e3_g = np.asarray(inp["e3_g"]); e3_be = np.asarray(inp["e3_be"])
    head_w = np.asarray(inp["head_w"]); head_b = np.asarray(inp["head_b"])

    if h == 1:
        f1 = f1[:, ::-1, :]
        f2 = f2[:, ::-1, :]
        flip = lambda w: w[:, :, ::-1, :]
        pre_w, fc1_w, fc2_w, e2_w, e3_w, head_w = map(
            flip, (pre_w, fc1_w, fc2_w, e2_w, e3_w, head_w))
        e1_w = flip(e1_w).copy()
        perm = np.array([(6 - i) * 7 + j for i in range(7) for j in range(7)])
        e1_w[:, 64:113] = e1_w[:, 64 + perm]

    def slab(x):
        s = np.zeros((CH, SLAB, WP), np.float32)
        s[:, R0:R0 + 146, X0:X0 + W] = x[:, 0:146, :]
        return s

    m0 = np.zeros((113, 1), np.float32); m0[0:64] = 1.0
    m1 = np.zeros((113, 1), np.float32)
    m1[64:113] = 1.0

    bd = np.stack([g * np.eye(64, dtype=np.float32) for g in GG])
    bdv = np.zeros((4, 128, 64), np.float32)
    for t in range(4):
        bdv[t, 0:64] = GG[2 * t] * np.eye(64, dtype=np.float32)
        if 2 * t + 1 < 7:
            bdv[t, 64:128] = GG[2 * t + 1] * np.eye(64, dtype=np.float32)
    bh128 = np.stack([g * np.eye(128, dtype=np.float32) for g in GG])
    bh112 = np.stack([g * np.eye(112, dtype=np.float32) for g in GG])

    bmv0 = _disp_vmat(18, 76, 18, 73, 18)
    bmv1 = _disp_vmat(71, 132, 74, 129, None)
    bmv2 = _disp_vmat(127, 148, 130, 145, None)

    # selector stationaries assembling lc = (A + B_shift - 2C)/64 in one psum:
    # C: 49 matmuls [128->98] with value -2/64
    cmat = np.zeros((49, 128, 98), np.float32)
    for k in range(49):
        cmat[k, 0:64, 2 * k] = -2.0 / 64.0
        cmat[k, 64:128, 2 * k + 1] = -2.0 / 64.0
    # B: 7 matmuls (one per horizontal shift j) [8 B-rows -> 98]
    bsel = np.zeros((7, 8, 98), np.float32)
    for i in range(7):
        for j in range(7):
            for q in range(2):
                bsel[j, i + q, 2 * (i * 7 + j) + q] = 1.0 / 64.0
    # A: 1 matmul [2 A-rows -> 98]
    asel = np.zeros((2, 98), np.float32)
    for k in range(49):
        for q in range(2):
            asel[q, 2 * k + q] = 1.0 / 64.0
    bd2 = np.zeros((128, 2), np.float32)
    bd2[0:64, 0] = 1.0
    bd2[64:128, 1] = 1.0
    zeros = np.zeros((128, 7 * WP), np.float32)

    d = {
        "feat1s": slab(f1), "feat2s": slab(f2),
        "preT": _tapT(pre_w), "pre_b": pre_b.reshape(64, 1),
        "fc1T": _tapT(fc1_w),
        "fc1_g": fc1_g.reshape(128, 1), "fc1_be": fc1_be.reshape(128, 1),
        "fc2T": _tapT(fc2_w), "fc2_b": fc2_b.reshape(64, 1),
        "e1T": _tapT(e1_w),
        "e1_g": e1_g.reshape(64, 1), "e1_be": e1_be.reshape(64, 1),
        "e2T": _tapT(e2_w), "e2_g": e2_g.reshape(32, 1), "e2_be": e2_be.reshape(32, 1),
        "e3T": _tapT(e3_w), "e3_g": e3_g.reshape(16, 1), "e3_be": e3_be.reshape(16, 1),
        "headT": _tapT(head_w), "head_b": head_b.reshape(2, 1),
        "m0": m0, "m1": m1,
        "bdv": bdv, "bd": bd, "bh128": bh128, "bh112": bh112, "cmat": cmat,
        "bsel": bsel, "asel": asel, "bd2": bd2, "zeros": zeros,
        "bmv0": bmv0, "bmv1": bmv1, "bmv2": bmv2,
    }
    return {k: np.ascontiguousarray(v, np.float32) for k, v in d.items()}


# ------------------------------------------------------------- wait fixer
# walrus in this container rejects instructions carrying more than a couple of
# sync waits; hoist excess waits onto single-wait NoOps in the same engine
# stream just before the instruction.
_SPLIT = {"InstDrain": 1, "InstMatmult": 0, "InstDMACopy": 1}
_SPLIT_DEFAULT = 1


def _fix_waits(nc):
    for fb in nc.m.functions[0].blocks:
        il = fb.instructions
        i = 0
        while i < len(il):
            inst = il[i]
            si = inst.sync_info
            mw = _SPLIT.get(type(inst).__name__, _SPLIT_DEFAULT)
            if si is not None and len(si.on_wait) > mw:
                ws = list(si.on_wait)
                si.on_wait = ws[:mw]
                inst.sync_info = si
                for j, wt in enumerate(ws[mw:]):
                    il.insert(i, mybir.InstNoOp(
                        name=f"{inst.name}-dw{j}", ins=[], outs=[],
                        engine=inst.engine, bass_nofuse=True,
                        sync_info=bass_rust.SyncInfo(on_wait=[wt], on_update=[])))
                    i += 1
            i += 1


def _pair_ap(buf, r):
    """Rows (r, r+1) of a [C, SLAB, WP] buf as a row-pair [2*64, WP] view."""
    return buf[:, r:r + 2, :].rearrange("c q x -> q c x")


# ------------------------------------------------------------- build
def build_module(collectives=True, upto=99):
    nc = bass.Bass(num_devices=8)

    def P(name, shape, dt=F32R):
        return nc.declare_dram_parameter(name, list(shape), dt, isOutput=False)

    f1in = P("feat1s", (CH, SLAB, WP))
    f2in = P("feat2s", (CH, SLAB, WP))
    preT = P("preT", (9, 64, 64)); pre_b = P("pre_b", (64, 1), F32)
    fc1T = P("fc1T", (9, 128, 128))
    fc1_g = P("fc1_g", (128, 1), F32); fc1_be = P("fc1_be", (128, 1), F32)
    fc2T = P("fc2T", (9, 128, 64)); fc2_b = P("fc2_b", (64, 1), F32)
    e1T = P("e1T", (9, 113, 64))
    e1_g = P("e1_g", (64, 1), F32); e1_be = P("e1_be", (64, 1), F32)
    e2T = P("e2T", (9, 64, 32))
    e2_g = P("e2_g", (32, 1), F32); e2_be = P("e2_be", (32, 1), F32)
    e3T = P("e3T", (9, 32, 16))
    e3_g = P("e3_g", (16, 1), F32); e3_be = P("e3_be", (16, 1), F32)
    headT = P("headT", (9, 16, 2)); head_b = P("head_b", (2, 1), F32)
    m0p = P("m0", (113, 1), F32); m1p = P("m1", (113, 1), F32)
    bdvp = P("bdv", (4, 128, 64)); bdp = P("bd", (7, 64, 64))
    bh128p = P("bh128", (7, 128, 128)); bh112p = P("bh112", (7, 112, 112))
    bmv0p = P("bmv0", (118, 112)); bmv1p = P("bmv1", (124, 112))
    bmv2p = P("bmv2", (44, 32))
    cmatp = P("cmat", (49, 128, 98))
    bselp = P("bsel", (7, 8, 98))
    aselp = P("asel", (2, 98))
    bd2p = P("bd2", (128, 2))
    zerop = P("zeros", (128, 7 * WP))

    out = nc.declare_dram_parameter("out", [2, HALF, W], F32, isOutput=True)

    f1b = nc.dram_tensor("f1b", [64, SLAB, WP], F32R)
    f2b = nc.dram_tensor("f2b", [64, SLAB, WP], F32R)
    t1b = nc.dram_tensor("t1b", [128, SLAB, WP], F32R)
    featb = nc.dram_tensor("featb", [64, SLAB, WP], F32R)
    f1sb = nc.dram_tensor("f1sb", [64, SLAB, WP], F32R)
    lcb = nc.dram_tensor("lcb", [49, SLAB, WP], F32R)
    bb = nc.dram_tensor("bb", [1, SLAB, WP], F32R)
    e1b = nc.dram_tensor("e1b", [64, SLAB, WP], F32R)
    e2b = nc.dram_tensor("e2b", [32, SLAB, WP], F32R)
    e3b = nc.dram_tensor("e3b", [16, SLAB, WP], F32R)
    headbuf = nc.dram_tensor("headbuf", [2, SLAB, WP], F32R)

    ar1i = nc.dram_tensor("ar1i", [1, 257], F32)
    ar1o = nc.dram_tensor("ar1o", [1, 257], F32)
    ar2i = nc.dram_tensor("ar2i", [1, 128], F32)
    ar2o = nc.dram_tensor("ar2o", [1, 128], F32)
    ar3i = nc.dram_tensor("ar3i", [1, 64], F32)
    ar3o = nc.dram_tensor("ar3o", [1, 64], F32)
    ar4i = nc.dram_tensor("ar4i", [1, 32], F32)
    ar4o = nc.dram_tensor("ar4o", [1, 32], F32)
    scr = nc.dram_tensor("scr", [1, 128], F32)

    with TileContext(nc) as tc, \
         tc.tile_pool(name="wpool", bufs=1) as wpool, \
         tc.tile_pool(name="cpool", bufs=1) as cpool, \
         tc.tile_pool(name="row", bufs=2) as rowp, \
         tc.tile_pool(name="big", bufs=4) as bigp, \
         tc.tile_pool(name="outp", bufs=3) as outp, \
         tc.tile_pool(name="stat", bufs=1) as statp, \
         tc.tile_pool(name="ps", bufs=4, space="PSUM") as psp, \
         tc.tile_pool(name="ps2", bufs=2, space="PSUM") as psp2:

        def wtile(src, shape, name, dt=F32R):
            t = wpool.tile(shape, dt, name=name)
            nc.sync.dma_start(out=t[:], in_=src)
            return t

        rr3 = lambda p: p[:, :, :].rearrange("t i o -> i t o")
        w_pre = wtile(rr3(preT), [64, 9 * 64], "w_pre")
        w_fc1 = wtile(rr3(fc1T), [128, 9 * 128], "w_fc1")
        w_fc2 = wtile(rr3(fc2T), [128, 9 * 64], "w_fc2")
        w_e1 = wtile(rr3(e1T), [113, 9 * 64], "w_e1")
        w_e2 = wtile(rr3(e2T), [64, 9 * 32], "w_e2")
        w_e3 = wtile(rr3(e3T), [32, 9 * 16], "w_e3")
        w_head = wtile(rr3(headT), [16, 9 * 2], "w_head")
        w_bdv = wtile(rr3(bdvp), [128, 4 * 64], "w_bdv")
        w_bd = wtile(rr3(bdp), [64, 7 * 64], "w_bd")
        w_bh128 = wtile(rr3(bh128p), [128, 7 * 128], "w_bh128")
        w_bh112 = wtile(rr3(bh112p), [112, 7 * 112], "w_bh112")
        w_bmv0 = wtile(bmv0p[:, :], [118, 112], "w_bmv0")
        w_bmv1 = wtile(bmv1p[:, :], [124, 112], "w_bmv1")
        w_bmv2 = wtile(bmv2p[:, :], [44, 32], "w_bmv2")
        w_cm = wtile(rr3(cmatp), [128, 49 * 98], "w_cm")
        w_bsel = wtile(bselp[:, :, :].rearrange("j u o -> u j o"), [8, 7 * 98], "w_bsel")
        w_asel = wtile(aselp[:, :], [2, 98], "w_asel")

        c_pre_b = wtile(pre_b[:, :], [64, 1], "c_pre_b", F32)
        c_fc1_g = wtile(fc1_g[:, :], [128, 1], "c_fc1_g", F32)
        c_fc1_be = wtile(fc1_be[:, :], [128, 1], "c_fc1_be", F32)
        c_fc2_b = wtile(fc2_b[:, :], [64, 1], "c_fc2_b", F32)
        c_e1_g = wtile(e1_g[:, :], [64, 1], "c_e1_g", F32)
        c_e1_be = wtile(e1_be[:, :], [64, 1], "c_e1_be", F32)
        c_e2_g = wtile(e2_g[:, :], [32, 1], "c_e2_g", F32)
        c_e2_be = wtile(e2_be[:, :], [32, 1], "c_e2_be", F32)
        c_e3_g = wtile(e3_g[:, :], [16, 1], "c_e3_g", F32)
        c_e3_be = wtile(e3_be[:, :], [16, 1], "c_e3_be", F32)
        c_head_b = wtile(head_b[:, :], [2, 1], "c_head_b", F32)
        c_m0 = wtile(m0p[:, :], [113, 1], "c_m0", F32)
        c_m1 = wtile(m1p[:, :], [113, 1], "c_m1", F32)

        c_bd2 = wtile(bd2p[:, :], [128, 2], "c_bd2")
        zt = wtile(zerop[:, :], [128, 7 * WP], "zt")

        for buf, c in [(f1b, 64), (f2b, 64), (t1b, 128), (featb, 64), (f1sb, 64),
                       (lcb, 49), (bb, 1), (e1b, 64), (e2b, 32),
                       (e3b, 16), (headbuf, 2)]:
            nc.sync.dma_start(out=buf[:, 11:18, :], in_=zt[:c, :7 * WP])
            nc.sync.dma_start(out=buf[:, :, 0:X0], in_=zt[:c, :SLAB * X0])
            nc.sync.dma_start(out=buf[:, :, WP - 4:WP], in_=zt[:c, :SLAB * 4])
        nc.sync.dma_start(out=f1b[:, 163:164, :], in_=zt[:64, :WP])

        st_fc1_s = statp.tile([128, 128], F32, name="st_fc1_s")
        st_fc1_q = statp.tile([128, 128], F32, name="st_fc1_q")
        st_e1_s = statp.tile([64, 128], F32, name="st_e1_s")
        st_e1_q = statp.tile([64, 128], F32, name="st_e1_q")
        st_e2_s = statp.tile([32, 128], F32, name="st_e2_s")
        st_e2_q = statp.tile([32, 128], F32, name="st_e2_q")
        st_e3_s = statp.tile([16, 128], F32, name="st_e3_s")
        st_e3_q = statp.tile([16, 128], F32, name="st_e3_q")
        st_c = statp.tile([98, 80], F32, name="st_c")
        for t in (st_fc1_s, st_fc1_q, st_e1_s, st_e1_q, st_e2_s, st_e2_q,
                  st_e3_s, st_e3_q, st_c):
            nc.vector.memset(t[:], 0.0)

        # ---------------- generic 3x3 conv pass ---------------------------
        def conv_pass(src_bufs, dst_buf, w_sb, cin, cout, dil, rows, tag,
                      bias=None, stats=None, norm=None):
            lo, hi = rows
            win = {}

            def load(r):
                t = rowp.tile([cin, WP], F32R, name=f"{tag}_in", tag="cin",
                              bufs=12)
                p = 0
                for bsrc, c in src_bufs:
                    nc.sync.dma_start(out=t[p:p + c, :], in_=bsrc[:, r, :])
                    p += c
                if norm is not None and r >= 18:
                    nc.scalar.activation(t[:, X0:X0 + W], t[:, X0:X0 + W], AF.Prelu,
                                         bias=norm[1], scale=norm[0], alpha=0.2)
                win[r] = t

            for r in range(lo - dil, lo + dil):
                load(r)
            for r in range(lo, hi + 1):
                load(r + dil)
                ps = psp.tile([cout, W], F32, name=f"{tag}_ps", tag="ps")
                for ti, (dy, dx) in enumerate(TAPS):
                    nc.tensor.matmul(
                        ps[:], w_sb[:, ti * cout:(ti + 1) * cout],
                        win[r + dy * dil][:, X0 + dx * dil:X0 + dx * dil + W],
                        start=(ti == 0), stop=(ti == 8))
                ot = outp.tile([cout, W], F32R, name=f"{tag}_o", tag="cout")
                acc = None
                if stats is not None and STAT_LO <= r <= STAT_HI:
                    acc = stats[0][:, r - STAT_LO:r - STAT_LO + 1]
                if bias is not None:
                    nc.scalar.activation(ot[:], ps[:], AF.Identity,
                                         bias=bias, accum_out=acc)
                else:
                    nc.scalar.activation(ot[:], ps[:], AF.Identity, accum_out=acc)
                if stats is not None and STAT_LO <= r <= STAT_HI:
                    sq = outp.tile([cout, W], F32, name=f"{tag}_sq", tag="csq")
                    nc.scalar.activation(
                        sq[:], ps[:], AF.Square,
                        accum_out=stats[1][:, r - STAT_LO:r - STAT_LO + 1])
                nc.sync.dma_start(out=dst_buf[:, r, X0:X0 + W], in_=ot[:])
                win.pop(r - dil, None)

        # ---------------- PASS 1: pre conv --------------------------------
        if upto >= 1:
            conv_pass([(f1in, 64)], f1b, w_pre, 64, 64, 1, ROWS_F1, "pre1",
                      bias=c_pre_b[:, 0:1])
            conv_pass([(f2in, 64)], f2b, w_pre, 64, 64, 1, ROWS_F1, "pre2",
                      bias=c_pre_b[:, 0:1])

        # ---------------- PASS 2: fc1 conv + stats ------------------------
        if upto >= 2:
            conv_pass([(f1b, 64), (f2b, 64)], t1b, w_fc1, 128, 128, 1, ROWS_T1, "fc1",
                      stats=(st_fc1_s, st_fc1_q))

        # ---------------- PASS 3: gaussian blur of f1, B ------------------
        if upto >= 3:
            fwin = {}

            def loadpair(r):
                t = rowp.tile([128, WP], F32R, name="f1p_in", tag="pin", bufs=11)
                nc.sync.dma_start(out=t[:], in_=_pair_ap(f1b, r))
                fwin[r] = t

            lo, hi = ROWS_F1S
            for r in range(lo, lo + 8):
                loadpair(r)
            vpair = None
            for r in range(lo, hi + 1):
                k = r + 3
                if k not in fwin and k <= 162:
                    loadpair(k)
                q = (r - lo) % 2
                if q == 0:
                    vpair = bigp.tile([128, WP], F32R, name="vpair", tag="bigA")
                    nc.vector.tensor_copy(vpair[:, 0:X0], zt[:, 0:X0])
                    nc.vector.tensor_copy(vpair[:, WP - 4:WP], zt[:, 0:4])
                psv = psp2.tile([64, W], F32, name="ps_v", tag="ps2")
                if r < lo + 3:
                    for d in range(7):
                        ri = max(r + d - 3, lo)
                        nc.tensor.matmul(psv[:], w_bd[:, d * 64:(d + 1) * 64],
                                         fwin[ri][0:64, X0:X0 + W],
                                         start=(d == 0), stop=(d == 6))
                else:
                    for t in range(4):
                        nc.tensor.matmul(psv[:], w_bdv[:, t * 64:(t + 1) * 64],
                                         fwin[r - 3 + 2 * t][:, X0:X0 + W],
                                         start=(t == 0), stop=(t == 3))
                nc.scalar.activation(vpair[q * 64:(q + 1) * 64, X0:X0 + W], psv[:],
                                     AF.Identity)
                if q == 1:
                    nc.vector.tensor_copy(vpair[:, 1:4],
                                          vpair[:, X0:X0 + 1].to_broadcast([128, 3]))
                    nc.vector.tensor_copy(vpair[:, WP - 4:WP - 1],
                                          vpair[:, X0 + W - 1:X0 + W].to_broadcast([128, 3]))
                    psh = psp.tile([128, W], F32, name="ps_h", tag="ps")
                    for j in range(7):
                        nc.tensor.matmul(psh[:], w_bh128[:, j * 128:(j + 1) * 128],
                                         vpair[:, 1 + j:1 + j + W],
                                         start=(j == 0), stop=(j == 6))
                    f1st = bigp.tile([128, W], F32R, name="f1st", tag="bigC", bufs=6)
                    nc.scalar.activation(f1st[:], psh[:], AF.Identity)
                    nc.sync.dma_start(
                        out=f1sb[:, r - 1:r + 1, X0:X0 + W].rearrange("c q x -> q c x"),
                        in_=f1st[:])
                    sqt = bigp.tile([128, W], F32R, name="sqB", tag="bigB")
                    nc.scalar.activation(sqt[:], psh[:], AF.Square)
                    psb = psp2.tile([2, W], F32, name="ps_b", tag="ps2")
                    nc.tensor.matmul(psb[:], c_bd2[:], sqt[:], start=True, stop=True)
                    bt = outp.tile([2, W], F32R, name="bt", tag="small")
                    nc.scalar.activation(bt[:], psb[:], AF.Identity)
                    nc.sync.dma_start(out=bb[0, r - 1:r + 1, X0:X0 + W], in_=bt[:])
                fwin.pop(r - 4, None)

        # ---------------- PASS 4: correlation -----------------------------
        if upto >= 4:
            swin = {}

            def loadspair(r):
                t = rowp.tile([128, WP], F32R, name="f1sp_in", tag="pin", bufs=11)
                nc.sync.dma_start(out=t[:], in_=_pair_ap(f1sb, r))
                swin[r] = t

            lo, hi = ROWS_CORR
            pair_starts = list(range(lo, 155, 2)) + [155]
            for r in range(lo - 3, lo + 4):
                loadspair(r)
            for pidx, r in enumerate(pair_starts):
                for rr in range(r - 3, r + 4):
                    if rr not in swin:
                        loadspair(rr)
                f2p = rowp.tile([128, WP], F32R, name="f2p", tag="aux", bufs=4)
                nc.sync.dma_start(out=f2p[:], in_=_pair_ap(f2b, r))
                sqa = bigp.tile([128, W], F32R, name="sqA", tag="bigB")
                nc.scalar.activation(sqa[:], f2p[:, X0:X0 + W], AF.Square)
                psa = psp2.tile([2, W], F32, name="ps_a", tag="ps2")
                nc.tensor.matmul(psa[:], c_bd2[:], sqa[:], start=True, stop=True)
                at = outp.tile([2, W], F32R, name="at", tag="small")
                nc.scalar.activation(at[:], psa[:], AF.Identity)
                bw8 = rowp.tile([8, WP], F32R, name="bw8", tag="aux", bufs=4)
                nc.sync.dma_start(out=bw8[:], in_=bb[0, r - 3:r + 5, :])
                psc = psp.tile([98, W], F32, name="ps_c", tag="ps")
                for i in range(7):
                    sp = swin[r + i - 3]
                    for j in range(7):
                        kk = i * 7 + j
                        prod = bigp.tile([128, W], F32R, name="prod", tag="bigC", bufs=6)
                        nc.vector.tensor_tensor(
                            prod[:], f2p[:, X0:X0 + W],
                            sp[:, X0 + j - 3:X0 + j - 3 + W], ALU.mult)
                        nc.tensor.matmul(psc[:], w_cm[:, kk * 98:(kk + 1) * 98],
                                         prod[:], start=(kk == 0), stop=False)
                for j in range(7):
                    nc.tensor.matmul(psc[:], w_bsel[:, j * 98:(j + 1) * 98],
                                     bw8[:, X0 + j - 3:X0 + j - 3 + W],
                                     start=False, stop=False)
                nc.tensor.matmul(psc[:], w_asel[:], at[:], start=False, stop=True)
                cred = outp.tile([98, 1], F32, name="cred", tag="small")
                nc.vector.tensor_reduce(cred[:], psc[:], AX.X, ALU.add)
                nc.vector.tensor_copy(st_c[:, pidx:pidx + 1], cred[:])
                ct = bigp.tile([98, W], F32R, name="ct", tag="bigA")
                nc.scalar.activation(ct[:], psc[:], AF.Identity)
                nc.sync.dma_start(out=lcb[:, r:r + 2, X0:X0 + W], in_=ct[:])
                for kk in (r - 3, r - 2):
                    swin.pop(kk, None)

        # ---------------- AR1 ---------------------------------------------
        if upto >= 5:
            fs = statp.tile([128, 1], F32, name="fs")
            fq = statp.tile([128, 1], F32, name="fq")
            nc.vector.tensor_reduce(fs[:], st_fc1_s[:], AX.X, ALU.add)
            nc.vector.tensor_reduce(fq[:], st_fc1_q[:], AX.X, ALU.add)
            credp = statp.tile([98, 1], F32, name="credp")
            nc.vector.tensor_reduce(credp[:], st_c[:, 0:64], AX.X, ALU.add)
            # C partial: flatten [98,1] -> [1,98] via DRAM, reduce to scalar
            nc.sync.dma_start(out=scr[0:1, 0:98], in_=credp[:])
            ctot_r = statp.tile([1, 98], F32, name="ctot_r")
            nc.sync.dma_start(out=ctot_r[:], in_=scr[0:1, 0:98])
            ctot = statp.tile([1, 1], F32, name="ctot")
            nc.vector.tensor_reduce(ctot[:], ctot_r[:], AX.X, ALU.add)
            # payload: [0:128) sum, [128:256) sumsq, 256 = lc total
            nc.sync.dma_start(out=ar1i[0:1, 0:128], in_=fs[:])
            nc.sync.dma_start(out=ar1i[0:1, 128:256], in_=fq[:])
            nc.sync.dma_start(out=ar1i[0:1, 256:257], in_=ctot[:])
            if collectives:
                nc.gpsimd.collective_compute("AllReduce", ALU.add, replica_groups=GROUPS,
                                             ins=[ar1i[:, :]], outs=[ar1o[:, :]])
            else:
                nc.sync.dma_start(out=ar1o[:, :], in_=ar1i[:, :])
            fc1s = statp.tile([128, 1], F32, name="fc1s")
            fc1q = statp.tile([128, 1], F32, name="fc1q")
            nc.sync.dma_start(out=fc1s[:], in_=ar1o[0:1, 0:128])
            nc.sync.dma_start(out=fc1q[:], in_=ar1o[0:1, 128:256])
            sc = statp.tile([1, 1], F32, name="sc")
            nc.sync.dma_start(out=sc[:], in_=ar1o[0:1, 256:257])

            def norm_params(ssum, sq, gam, bet, n, c, nm):
                mu = statp.tile([c, 1], F32, name=f"mu_{nm}")
                nc.vector.tensor_scalar_mul(mu[:], ssum[:], 1.0 / n)
                var = statp.tile([c, 1], F32, name=f"var_{nm}")
                nc.vector.tensor_scalar_mul(var[:], sq[:], 1.0 / n)
                mu2 = statp.tile([c, 1], F32, name=f"mu2_{nm}")
                nc.vector.tensor_tensor(mu2[:], mu[:], mu[:], ALU.mult)
                nc.vector.tensor_tensor(var[:], var[:], mu2[:], ALU.subtract)
                nc.vector.tensor_scalar_add(var[:], var[:], EPS_IN)
                sd = statp.tile([c, 1], F32, name=f"sd_{nm}")
                nc.scalar.activation(sd[:], var[:], AF.Sqrt)
                rstd = statp.tile([c, 1], F32, name=f"rstd_{nm}")
                nc.vector.reciprocal(rstd[:], sd[:])
                s_ = statp.tile([c, 1], F32, name=f"s_{nm}")
                nc.vector.tensor_tensor(s_[:], gam[:], rstd[:], ALU.mult)
                b_ = statp.tile([c, 1], F32, name=f"b_{nm}")
                nc.vector.tensor_tensor(b_[:], mu[:], s_[:], ALU.mult)
                nc.vector.tensor_tensor(b_[:], bet[:], b_[:], ALU.subtract)
                return s_[:, 0:1], b_[:, 0:1]

            fc1_sn, fc1_bn = norm_params(fc1s, fc1q, c_fc1_g, c_fc1_be, 65536.0, 128, "fc1")

            # s = lcT / (49*H*W) + eps ; rs = 1/s ; scale e1 lc-channel weights
            sval = statp.tile([1, 1], F32, name="sval")
            nc.vector.tensor_scalar(sval[:], sc[0:1, 0:1], 1.0 / (49.0 * H * W), None,
                                    ALU.mult)
            nc.vector.tensor_scalar_add(sval[:], sval[:], EPS_CORR)
            rsv = statp.tile([1, 1], F32, name="rsv")
            nc.vector.reciprocal(rsv[:], sval[:])
            nc.sync.dma_start(out=scr[0:1, 100:101], in_=rsv[:])
            rb113 = statp.tile([113, 1], F32, name="rb113")
            nc.sync.dma_start(out=rb113[:],
                              in_=bass.AP(scr, 100, [[0, 113], [1, 1]]))
            fac = statp.tile([113, 1], F32, name="fac")
            nc.vector.tensor_tensor(fac[:], c_m1[:], rb113[:], ALU.mult)
            nc.vector.tensor_tensor(fac[:], fac[:], c_m0[:], ALU.add)
            nc.vector.tensor_scalar(w_e1[:], w_e1[:], fac[:, 0:1], None, ALU.mult)

        # ---------------- PASS 5: fc2 -------------------------------------
        if upto >= 6:
            conv_pass([(t1b, 128)], featb, w_fc2, 128, 64, 1, ROWS_FEAT, "fc2",
                      bias=c_fc2_b[:, 0:1], norm=(fc1_sn, fc1_bn))

        # ---------------- PASS 6: e1 --------------------------------------
        if upto >= 7:
            lo, hi = ROWS_E1
            ewin = {}

            def eload(r):
                t = rowp.tile([113, WP], F32R, name="e1_in", tag="cin", bufs=12)
                nc.sync.dma_start(out=t[0:64, :], in_=featb[:, r, :])
                nc.sync.dma_start(out=t[64:113, :], in_=lcb[:, r, :])
                ewin[r] = t

            for r in range(lo - 1, lo + 1):
                eload(r)
            for r in range(lo, hi + 1):
                eload(r + 1)
                ps = psp.tile([64, W], F32, name="e1_ps", tag="ps")
                for ti, (dy, dx) in enumerate(TAPS):
                    nc.tensor.matmul(ps[:], w_e1[:, ti * 64:(ti + 1) * 64],
                                     ewin[r + dy][:, X0 + dx:X0 + dx + W],
                                     start=(ti == 0), stop=(ti == 8))
                ot = outp.tile([64, W], F32R, name="e1_o", tag="cout")
                acc = st_e1_s[:, r - STAT_LO:r - STAT_LO + 1] if STAT_LO <= r <= STAT_HI else None
                nc.scalar.activation(ot[:], ps[:], AF.Identity, accum_out=acc)
                if STAT_LO <= r <= STAT_HI:
                    sq = outp.tile([64, W], F32, name="e1_sq", tag="csq")
                    nc.scalar.activation(sq[:], ps[:], AF.Square,
                                         accum_out=st_e1_q[:, r - STAT_LO:r - STAT_LO + 1])
                nc.sync.dma_start(out=e1b[:, r, X0:X0 + W], in_=ot[:])
                ewin.pop(r - 1, None)

        # ---- AR2 / e2 / AR3 / e3 / AR4 / head ---------------------------
        if upto >= 8:
            def stats_ar(st_s, st_q, c, ari, aro, nm):
                s1 = statp.tile([c, 1], F32, name=f"ss_{nm}")
                q1 = statp.tile([c, 1], F32, name=f"qq_{nm}")
                nc.vector.tensor_reduce(s1[:], st_s[:], AX.X, ALU.add)
                nc.vector.tensor_reduce(q1[:], st_q[:], AX.X, ALU.add)
                nc.sync.dma_start(out=ari[0:1, 0:c], in_=s1[:])
                nc.sync.dma_start(out=ari[0:1, c:2 * c], in_=q1[:])
                if collectives:
                    nc.gpsimd.collective_compute("AllReduce", ALU.add,
                                                 replica_groups=GROUPS,
                                                 ins=[ari[:, :]], outs=[aro[:, :]])
                else:
                    nc.sync.dma_start(out=aro[:, :], in_=ari[:, :])
                s2 = statp.tile([c, 1], F32, name=f"ss2_{nm}")
                q2 = statp.tile([c, 1], F32, name=f"qq2_{nm}")
                nc.sync.dma_start(out=s2[:], in_=aro[0:1, 0:c])
                nc.sync.dma_start(out=q2[:], in_=aro[0:1, c:2 * c])
                return s2, q2

            e1s2, e1q2 = stats_ar(st_e1_s, st_e1_q, 64, ar2i, ar2o, "e1")
            e1_sn, e1_bn = norm_params(e1s2, e1q2, c_e1_g, c_e1_be, 65536.0, 64, "e1")

            conv_pass([(e1b, 64)], e2b, w_e2, 64, 32, 2, ROWS_E2, "e2",
                      stats=(st_e2_s, st_e2_q), norm=(e1_sn, e1_bn))
            e2s2, e2q2 = stats_ar(st_e2_s, st_e2_q, 32, ar3i, ar3o, "e2")
            e2_sn, e2_bn = norm_params(e2s2, e2q2, c_e2_g, c_e2_be, 65536.0, 32, "e2")

            conv_pass([(e2b, 32)], e3b, w_e3, 32, 16, 4, ROWS_E3, "e3",
                      stats=(st_e3_s, st_e3_q), norm=(e2_sn, e2_bn))
            e3s2, e3q2 = stats_ar(st_e3_s, st_e3_q, 16, ar4i, ar4o, "e3")
            e3_sn, e3_bn = norm_params(e3s2, e3q2, c_e3_g, c_e3_be, 65536.0, 16, "e3")

            conv_pass([(e3b, 16)], headbuf, w_head, 16, 2, 1, ROWS_HEAD, "head",
                      bias=c_head_b[:, 0:1], norm=(e3_sn, e3_bn))

        # ---------------- PASS 10: disp blur ------------------------------
        if upto >= 9:
            blocks = [
                (w_bmv0, 118, 18, 59, 18, 56, 112),
                (w_bmv1, 124, 71, 62, 74, 56, 112),
                (w_bmv2, 44, 127, 22, 130, 16, 32),
            ]
            for wm, kp, rin0, nin, ro0, nout, mp in blocks:
                ht = bigp.tile([kp, WP], F32R, name="dh_in", tag="bigA")
                nc.sync.dma_start(
                    out=ht[:],
                    in_=headbuf[:, rin0:rin0 + nin, :].rearrange("c r x -> r c x"))
                psv = psp.tile([mp, WP], F32, name="dps_v", tag="ps")
                nc.tensor.matmul(psv[:], wm[:], ht[:], start=True, stop=True)
                vt = bigp.tile([mp, WP], F32R, name="dvt", tag="bigB")
                nc.scalar.activation(vt[:], psv[:], AF.Identity)
                nc.vector.tensor_copy(vt[:, 1:4], vt[:, X0:X0 + 1].to_broadcast([mp, 3]))
                nc.vector.tensor_copy(vt[:, WP - 4:WP - 1],
                                      vt[:, X0 + W - 1:X0 + W].to_broadcast([mp, 3]))
                psh = psp.tile([mp, W], F32, name="dps_h", tag="ps")
                for j in range(7):
                    nc.tensor.matmul(psh[:], w_bh112[0:mp, j * 112:j * 112 + mp],
                                     vt[:, 1 + j:1 + j + W], start=(j == 0), stop=(j == 6))
                ot = outp.tile([mp, W], F32, name="do", tag="small")
                nc.scalar.activation(ot[:], psh[:], AF.Identity)
                nc.sync.dma_start(
                    out=out[:, ro0 - R0:ro0 - R0 + nout, :].rearrange("c r x -> r c x"),
                    in_=ot[:])

    _fix_waits(nc)
    return nc


# ---------------------------------------------------------------- entry
_NC = None


def kernel(**inputs):
    global _NC
    if _NC is None:
        _NC = build_module()
    in_maps = [_prep_core_inputs(inputs, c) for c in range(8)]
    res = run_bass_kernel_spmd(_NC, in_maps, list(range(8)))
    disp = np.zeros((B, 2, H, W), np.float32)
    for c in range(8):
        b, h = c // 2, c % 2
        o = res.results[c]["out"]
        if h == 0:
            disp[b, :, 0:HALF, :] = o
        else:
            disp[b, :, HALF:H, :] = o[:, ::-1, :]
    return disp



# revision 13
# speedup vs baseline: 1.2087x; 1.2087x over previous
"""DispEstimator Trainium2 kernel: 8-core SPMD (batch x H-half sharding).

Core c handles sample b=c//2, vertical half h=c%2. Odd cores get a vertically
flipped view of their sample (dy-flipped weights, permuted lc-channel order
for e1), so every core runs identical code: the "top" slab edge is a true
image boundary, the "bottom" edge is interior halo. Instance-norm and
correlation-normalization statistics are made global via pairwise AllReduce
between the two cores of each sample.

Block-processing layout (v2): every pass works on 8-row blocks with one DMA
per multi-row transfer (the shared HWDGE descriptor queue charges a fixed
~625ns per DMA instruction, so DMA count, not bytes, is the scarce resource).
Matmuls stream 2 rows per instruction (free dims [2, 256] = 512 = one PSUM
bank). The gaussian blur and the correlation run in a row-pair layout
([2*64, ...] partitions) so the vertical 7-tap blur needs only 4 matmuls per
2 output pairs and the correlation reduces both pair rows in one pass.

Correlation: lc_k = (A + B_shift - 2*C_k)/64 with A = sum_c f2^2,
B = sum_c f1s^2, C_k = sum_c f2 * shift_k(f1s). C is computed on
DVE (products) + TensorE (block-ones reduction, q-major psum layout). The
1/(64*s) normalization is baked into the e1 weight tiles after AllReduce.
"""
import sys

import numpy as np

if "/opt/trn_rl_repo" not in sys.path:
    sys.path.insert(0, "/opt/trn_rl_repo")

import bass_rust
import concourse.bass as bass
import concourse.mybir as mybir
from concourse.bass_utils import run_bass_kernel_spmd
from concourse.tile import TileContext

F32 = mybir.dt.float32
F32R = mybir.dt.float32r
AF = mybir.ActivationFunctionType
ALU = mybir.AluOpType
AX = mybir.AxisListType

B, CH, H, W = 4, 64, 256, 256
HALF = 128
HALO = 18
SLAB = HALO + HALF + HALO  # 164
WP = W + 8                 # 264, data at cols X0..X0+255
X0 = 4
R0 = HALO                  # slab row of image row 0
BLK = 8

EPS_CORR = 1e-6
EPS_IN = 1e-5

ROWS_F1 = (18, 162)
ROWS_T1 = (18, 161)
ROWS_FEAT = (18, 160)
ROWS_F1S = (18, 159)
ROWS_CORR = (18, 156)
ROWS_E1 = (18, 155)
ROWS_E2 = (18, 153)
ROWS_E3 = (18, 149)
ROWS_HEAD = (18, 148)
STAT_LO, STAT_HI = 18, 145

GROUPS = [[0, 1], [2, 3], [4, 5], [6, 7]]
TAPS = [(dy - 1, dx - 1) for dy in range(3) for dx in range(3)]


def _gauss1d():
    x = np.arange(7, dtype=np.float32) - 3.0
    g = np.exp(-(x ** 2) / (2.0 * 1.5 ** 2))
    return (g / g.sum()).astype(np.float32)


GG = _gauss1d()


# ---------------------------------------------------------------- host prep
def _tapT(w):
    """[O, I, 3, 3] -> [9, I, O] tap-major stationary layout."""
    o, i, _, _ = w.shape
    return np.ascontiguousarray(w.transpose(2, 3, 1, 0).reshape(9, i, o))


def _disp_vmat(rin_lo, rin_hi, ro_lo, ro_hi, clamp_lo):
    nin = rin_hi - rin_lo + 1
    nout = ro_hi - ro_lo + 1
    m = np.zeros((2 * nin, 2 * nout), np.float32)
    for ro in range(ro_lo, ro_hi + 1):
        for d in range(7):
            ri = ro + d - 3
            if clamp_lo is not None:
                ri = max(ri, clamp_lo)
            assert rin_lo <= ri <= rin_hi, (ro, d, ri)
            for c in range(2):
                m[2 * (ri - rin_lo) + c, 2 * (ro - ro_lo) + c] += GG[d]
    return m


def _prep_core_inputs(inp, core):
    b, h = core // 2, core % 2
    f1 = np.asarray(inp["feat1"][b], np.float32)
    f2 = np.asarray(inp["feat2"][b], np.float32)
    pre_w = np.asarray(inp["pre_w"]); pre_b = np.asarray(inp["pre_b"])
    fc1_w = np.asarray(inp["fc1_w"]); fc1_g = np.asarray(inp["fc1_g"]); fc1_be = np.asarray(inp["fc1_be"])
    fc2_w = np.asarray(inp["fc2_w"]); fc2_b = np.asarray(inp["fc2_b"])
    e1_w = np.asarray(inp["e1_w"]); e1_g = np.asarray(inp["e1_g"]); e1_be = np.asarray(inp["e1_be"])
    e2_w = np.asarray(inp["e2_w"]); e2_g = np.asarray(inp["e2_g"]); e2_be = np.asarray(inp["e2_be"])
    e3_w = np.asarray(inp["e3_w"]); e3_g = np.asarray(inp["e3_g"]); e3_be = np.asarray(inp["e3_be"])
    head_w = np.asarray(inp["head_w"]); head_b = np.asarray(inp["head_b"])

    if h == 1:
        f1 = f1[:, ::-1, :]
        f2 = f2[:, ::-1, :]
        flip = lambda w: w[:, :, ::-1, :]
        pre_w, fc1_w, fc2_w, e2_w, e3_w, head_w = map(
            flip, (pre_w, fc1_w, fc2_w, e2_w, e3_w, head_w))
        e1_w = flip(e1_w).copy()
        perm = np.array([(6 - i) * 7 + j for i in range(7) for j in range(7)])
        e1_w[:, 64:113] = e1_w[:, 64 + perm]

    def slab(x):
        s = np.zeros((CH, SLAB, WP), np.float32)
        s[:, R0:R0 + 146, X0:X0 + W] = x[:, 0:146, :]
        return s

    m0 = np.zeros((113, 1), np.float32); m0[0:64] = 1.0
    m1 = np.zeros((113, 1), np.float32)
    m1[64:113] = 1.0

    # vertical-blur pair stationaries: moving = pair (start, start+1) in
    # partitions (q*64+c); out = 2 output pairs in free dim. Slot for
    # stationary t and out-pair j is s = 2*(t+j) in an all-parity tile.
    vst = np.zeros((4, 128, 128), np.float32)
    for t in range(4):
        for q in range(2):
            for jp in range(2):
                d = 2 * t + q - jp
                if 0 <= d <= 6:
                    for c in range(64):
                        vst[t, q * 64 + c, jp * 64 + c] = GG[d]

    bd = np.stack([g * np.eye(64, dtype=np.float32) for g in GG])
    bh128 = np.stack([g * np.eye(128, dtype=np.float32) for g in GG])
    bh112 = np.stack([g * np.eye(112, dtype=np.float32) for g in GG])

    bmv0 = _disp_vmat(18, 76, 18, 73, 18)
    bmv1 = _disp_vmat(71, 132, 74, 129, None)
    bmv2 = _disp_vmat(127, 148, 130, 145, None)

    # selector stationaries assembling lc = (A + B_shift - 2C)/64 in one psum.
    # q-major psum layout: col = q*49 + k so the store can split by q into
    # contiguous partition halves.
    cmat = np.zeros((49, 128, 98), np.float32)
    for k in range(49):
        cmat[k, 0:64, k] = -2.0 / 64.0
        cmat[k, 64:128, 49 + k] = -2.0 / 64.0
    # B: one K-packed matmul; partition = (j*8 + iu) of a [56, npair, 528]
    # window tile; B row for out row (p, q) at shift (i, j) is iu = i + q.
    bsel56 = np.zeros((56, 98), np.float32)
    for j in range(7):
        for i in range(7):
            for q in range(2):
                bsel56[j * 8 + (i + q), q * 49 + i * 7 + j] = 1.0 / 64.0
    asel = np.zeros((2, 98), np.float32)
    for k in range(49):
        for q in range(2):
            asel[q, q * 49 + k] = 1.0 / 64.0
    bd2 = np.zeros((128, 2), np.float32)
    bd2[0:64, 0] = 1.0
    bd2[64:128, 1] = 1.0
    zeros = np.zeros((128, 7 * WP), np.float32)

    d = {
        "feat1s": slab(f1), "feat2s": slab(f2),
        "preT": _tapT(pre_w), "pre_b": pre_b.reshape(64, 1),
        "fc1T": _tapT(fc1_w),
        "fc1_g": fc1_g.reshape(128, 1), "fc1_be": fc1_be.reshape(128, 1),
        "fc2T": _tapT(fc2_w), "fc2_b": fc2_b.reshape(64, 1),
        "e1T": _tapT(e1_w),
        "e1_g": e1_g.reshape(64, 1), "e1_be": e1_be.reshape(64, 1),
        "e2T": _tapT(e2_w), "e2_g": e2_g.reshape(32, 1), "e2_be": e2_be.reshape(32, 1),
        "e3T": _tapT(e3_w), "e3_g": e3_g.reshape(16, 1), "e3_be": e3_be.reshape(16, 1),
        "headT": _tapT(head_w), "head_b": head_b.reshape(2, 1),
        "m0": m0, "m1": m1,
        "vst": vst, "bd": bd, "bh128": bh128, "bh112": bh112, "cmat": cmat,
        "bsel56": bsel56, "asel": asel, "bd2": bd2, "zeros": zeros,
        "bmv0": bmv0, "bmv1": bmv1, "bmv2": bmv2,
    }
    return {k: np.ascontiguousarray(v, np.float32) for k, v in d.items()}


# ------------------------------------------------------------- wait fixer
# walrus in this container rejects instructions carrying more than a couple of
# sync waits; hoist excess waits onto single-wait NoOps in the same engine
# stream just before the instruction.
_SPLIT = {"InstDrain": 1, "InstMatmult": 0, "InstDMACopy": 1}
_SPLIT_DEFAULT = 1


def _fix_waits(nc):
    for fb in nc.m.functions[0].blocks:
        il = fb.instructions
        i = 0
        while i < len(il):
            inst = il[i]
            si = inst.sync_info
            mw = _SPLIT.get(type(inst).__name__, _SPLIT_DEFAULT)
            if si is not None and len(si.on_wait) > mw:
                ws = list(si.on_wait)
                si.on_wait = ws[:mw]
                inst.sync_info = si
                for j, wt in enumerate(ws[mw:]):
                    il.insert(i, mybir.InstNoOp(
                        name=f"{inst.name}-dw{j}", ins=[], outs=[],
                        engine=inst.engine, bass_nofuse=True,
                        sync_info=bass_rust.SyncInfo(on_wait=[wt], on_update=[])))
                    i += 1
            i += 1


# ------------------------------------------------------------- build
def build_module(collectives=True, upto=99):
    nc = bass.Bass(num_devices=8)

    def P(name, shape, dt=F32R):
        return nc.declare_dram_parameter(name, list(shape), dt, isOutput=False)

    f1in = P("feat1s", (CH, SLAB, WP))
    f2in = P("feat2s", (CH, SLAB, WP))
    preT = P("preT", (9, 64, 64)); pre_b = P("pre_b", (64, 1), F32)
    fc1T = P("fc1T", (9, 128, 128))
    fc1_g = P("fc1_g", (128, 1), F32); fc1_be = P("fc1_be", (128, 1), F32)
    fc2T = P("fc2T", (9, 128, 64)); fc2_b = P("fc2_b", (64, 1), F32)
    e1T = P("e1T", (9, 113, 64))
    e1_g = P("e1_g", (64, 1), F32); e1_be = P("e1_be", (64, 1), F32)
    e2T = P("e2T", (9, 64, 32))
    e2_g = P("e2_g", (32, 1), F32); e2_be = P("e2_be", (32, 1), F32)
    e3T = P("e3T", (9, 32, 16))
    e3_g = P("e3_g", (16, 1), F32); e3_be = P("e3_be", (16, 1), F32)
    headT = P("headT", (9, 16, 2)); head_b = P("head_b", (2, 1), F32)
    m0p = P("m0", (113, 1), F32); m1p = P("m1", (113, 1), F32)
    vstp = P("vst", (4, 128, 128)); bdp = P("bd", (7, 64, 64))
    bh128p = P("bh128", (7, 128, 128)); bh112p = P("bh112", (7, 112, 112))
    bmv0p = P("bmv0", (118, 112)); bmv1p = P("bmv1", (124, 112))
    bmv2p = P("bmv2", (44, 32))
    cmatp = P("cmat", (49, 128, 98))
    bsel56p = P("bsel56", (56, 98))
    aselp = P("asel", (2, 98))
    bd2p = P("bd2", (128, 2))
    zerop = P("zeros", (128, 7 * WP))

    out = nc.declare_dram_parameter("out", [2, HALF, W], F32, isOutput=True)

    f1b = nc.dram_tensor("f1b", [64, SLAB, WP], F32R)
    f2b = nc.dram_tensor("f2b", [64, SLAB, WP], F32R)
    t1b = nc.dram_tensor("t1b", [128, SLAB, WP], F32R)
    featb = nc.dram_tensor("featb", [64, SLAB, WP], F32R)
    f1sb = nc.dram_tensor("f1sb", [64, SLAB, WP], F32R)
    lcb = nc.dram_tensor("lcb", [49, SLAB, WP], F32R)
    bb = nc.dram_tensor("bb", [1, SLAB, WP], F32R)
    e1b = nc.dram_tensor("e1b", [64, SLAB, WP], F32R)
    e2b = nc.dram_tensor("e2b", [32, SLAB, WP], F32R)
    e3b = nc.dram_tensor("e3b", [16, SLAB, WP], F32R)
    headbuf = nc.dram_tensor("headbuf", [2, SLAB, WP], F32R)

    ar1i = nc.dram_tensor("ar1i", [1, 257], F32)
    ar1o = nc.dram_tensor("ar1o", [1, 257], F32)
    ar2i = nc.dram_tensor("ar2i", [1, 128], F32)
    ar2o = nc.dram_tensor("ar2o", [1, 128], F32)
    ar3i = nc.dram_tensor("ar3i", [1, 64], F32)
    ar3o = nc.dram_tensor("ar3o", [1, 64], F32)
    ar4i = nc.dram_tensor("ar4i", [1, 32], F32)
    ar4o = nc.dram_tensor("ar4o", [1, 32], F32)
    scr = nc.dram_tensor("scr", [1, 128], F32)

    with TileContext(nc) as tc, \
         tc.tile_pool(name="wpool", bufs=1) as wpool, \
         tc.tile_pool(name="row", bufs=2) as rowp, \
         tc.tile_pool(name="big", bufs=4) as bigp, \
         tc.tile_pool(name="outp", bufs=3) as outp, \
         tc.tile_pool(name="stat", bufs=1) as statp, \
         tc.tile_pool(name="ps", bufs=4, space="PSUM") as psp, \
         tc.tile_pool(name="ps2", bufs=2, space="PSUM") as psp2:

        def wtile(src, shape, name, dt=F32R):
            t = wpool.tile(shape, dt, name=name)
            nc.sync.dma_start(out=t[:], in_=src)
            return t

        rr3 = lambda p: p[:, :, :].rearrange("t i o -> i t o")
        w_pre = wtile(rr3(preT), [64, 9 * 64], "w_pre")
        w_fc1 = wtile(rr3(fc1T), [128, 9 * 128], "w_fc1")
        w_fc2 = wtile(rr3(fc2T), [128, 9 * 64], "w_fc2")
        w_e1 = wtile(rr3(e1T), [113, 9 * 64], "w_e1")
        w_e2 = wtile(rr3(e2T), [64, 9 * 32], "w_e2")
        w_e3 = wtile(rr3(e3T), [32, 9 * 16], "w_e3")
        w_head = wtile(rr3(headT), [16, 9 * 2], "w_head")
        w_vst = wtile(rr3(vstp), [128, 4 * 128], "w_vst")
        w_bd = wtile(rr3(bdp), [64, 7 * 64], "w_bd")
        w_bh128 = wtile(rr3(bh128p), [128, 7 * 128], "w_bh128")
        w_bh112 = wtile(rr3(bh112p), [112, 7 * 112], "w_bh112")
        w_bmv0 = wtile(bmv0p[:, :], [118, 112], "w_bmv0")
        w_bmv1 = wtile(bmv1p[:, :], [124, 112], "w_bmv1")
        w_bmv2 = wtile(bmv2p[:, :], [44, 32], "w_bmv2")
        w_cm = wtile(rr3(cmatp), [128, 49 * 98], "w_cm")
        w_bsel56 = wtile(bsel56p[:, :], [56, 98], "w_bsel56")
        w_asel = wtile(aselp[:, :], [2, 98], "w_asel")

        c_pre_b = wtile(pre_b[:, :], [64, 1], "c_pre_b", F32)
        c_fc1_g = wtile(fc1_g[:, :], [128, 1], "c_fc1_g", F32)
        c_fc1_be = wtile(fc1_be[:, :], [128, 1], "c_fc1_be", F32)
        c_fc2_b = wtile(fc2_b[:, :], [64, 1], "c_fc2_b", F32)
        c_e1_g = wtile(e1_g[:, :], [64, 1], "c_e1_g", F32)
        c_e1_be = wtile(e1_be[:, :], [64, 1], "c_e1_be", F32)
        c_e2_g = wtile(e2_g[:, :], [32, 1], "c_e2_g", F32)
        c_e2_be = wtile(e2_be[:, :], [32, 1], "c_e2_be", F32)
        c_e3_g = wtile(e3_g[:, :], [16, 1], "c_e3_g", F32)
        c_e3_be = wtile(e3_be[:, :], [16, 1], "c_e3_be", F32)
        c_head_b = wtile(head_b[:, :], [2, 1], "c_head_b", F32)
        c_m0 = wtile(m0p[:, :], [113, 1], "c_m0", F32)
        c_m1 = wtile(m1p[:, :], [113, 1], "c_m1", F32)
        c_bd2 = wtile(bd2p[:, :], [128, 2], "c_bd2")

        for buf, c in [(f1b, 64), (f2b, 64), (t1b, 128), (featb, 64), (f1sb, 64),
                       (lcb, 49), (bb, 1), (e1b, 64), (e2b, 32),
                       (e3b, 16), (headbuf, 2)]:
            nc.sync.dma_start(out=buf[:, 11:18, :], in_=zerop[:c, :7 * WP])
            nc.sync.dma_start(out=buf[:, :, 0:X0], in_=zerop[:c, :SLAB * X0])
            nc.sync.dma_start(out=buf[:, :, WP - 4:WP], in_=zerop[:c, :SLAB * 4])
        nc.sync.dma_start(out=bb[:, 160:164, :], in_=zerop[0:1, :4 * WP])

        st_fc1_s = statp.tile([128, 64], F32, name="st_fc1_s")
        st_fc1_q = statp.tile([128, 64], F32, name="st_fc1_q")
        st_e1_s = statp.tile([64, 64], F32, name="st_e1_s")
        st_e1_q = statp.tile([64, 64], F32, name="st_e1_q")
        st_e2_s = statp.tile([32, 64], F32, name="st_e2_s")
        st_e2_q = statp.tile([32, 64], F32, name="st_e2_q")
        st_e3_s = statp.tile([16, 64], F32, name="st_e3_s")
        st_e3_q = statp.tile([16, 64], F32, name="st_e3_q")
        st_c = statp.tile([98, 32], F32, name="st_c")
        for t in (st_fc1_s, st_fc1_q, st_e1_s, st_e1_q, st_e2_s, st_e2_q,
                  st_e3_s, st_e3_q, st_c):
            nc.vector.memset(t[:], 0.0)

        # ---------------- generic blocked 3x3 conv pass -------------------
        def conv_pass(src_bufs, dst_buf, w_sb, cin, cout, dil, rows, tag,
                      bias=None, stats=None, norm=None):
            lo, hi = rows
            blk = 4 if dil == 4 else BLK
            for a in range(lo, hi + 1, blk):
                nb = min(blk, hi + 1 - a)
                rin0 = a - dil
                cnt = nb + 2 * dil
                t = rowp.tile([cin, BLK + 4, WP], F32R,
                              name=f"{tag}_in", tag="cin", bufs=2)
                p = 0
                for bsrc, c in src_bufs:
                    nc.sync.dma_start(out=t[p:p + c, 0:cnt, :],
                                      in_=bsrc[:, rin0:rin0 + cnt, :])
                    p += c
                if norm is not None:
                    s0 = max(0, 18 - rin0)
                    nc.scalar.activation(t[:, s0:cnt, X0:X0 + W],
                                         t[:, s0:cnt, X0:X0 + W],
                                         AF.Prelu, bias=norm[1], scale=norm[0],
                                         alpha=0.2)
                ot = outp.tile([cout, BLK, W], F32R, name=f"{tag}_o",
                               tag="cout", bufs=3)  # blk<=BLK rows used
                for c0 in range(a, a + nb, 2):
                    ncr = min(2, a + nb - c0)
                    ps = psp.tile([cout, 2, W], F32, name=f"{tag}_ps", tag="ps")
                    for ti, (dy, dx) in enumerate(TAPS):
                        base = (c0 - a) + dil * (1 + dy)
                        nc.tensor.matmul(
                            ps[:, 0:ncr, :], w_sb[:, ti * cout:(ti + 1) * cout],
                            t[:, base:base + ncr, X0 + dx * dil:X0 + dx * dil + W],
                            start=(ti == 0), stop=(ti == 8))
                    osl = ot[:, c0 - a:c0 - a + ncr, :]
                    acc = None
                    if stats is not None and c0 >= STAT_LO and c0 + ncr - 1 <= STAT_HI:
                        col = (c0 - STAT_LO) // 2
                        acc = stats[0][:, col:col + 1]
                    if bias is not None:
                        nc.scalar.activation(osl, ps[:, 0:ncr, :], AF.Identity,
                                             bias=bias, accum_out=acc)
                    else:
                        nc.scalar.activation(osl, ps[:, 0:ncr, :], AF.Identity,
                                             accum_out=acc)
                    if acc is not None:
                        sq = bigp.tile([cout, 2, W], F32, name=f"{tag}_sq",
                                       tag="csq", bufs=1)
                        nc.scalar.activation(sq[:, 0:ncr, :], ps[:, 0:ncr, :],
                                             AF.Square,
                                             accum_out=stats[1][:, col:col + 1])
                nc.scalar.dma_start(out=dst_buf[:, a:a + nb, X0:X0 + W],
                                    in_=ot[:, 0:nb, :])

        # ---------------- PASS 1: pre conv --------------------------------
        if upto >= 1:
            conv_pass([(f1in, 64)], f1b, w_pre, 64, 64, 1, ROWS_F1, "pre1",
                      bias=c_pre_b[:, 0:1])
            conv_pass([(f2in, 64)], f2b, w_pre, 64, 64, 1, ROWS_F1, "pre2",
                      bias=c_pre_b[:, 0:1])

        # ---------------- PASS 2: fc1 conv + stats ------------------------
        if upto >= 2:
            conv_pass([(f1b, 64), (f2b, 64)], t1b, w_fc1, 128, 128, 1, ROWS_T1,
                      "fc1", stats=(st_fc1_s, st_fc1_q))

        # ---------------- PASS 3: gaussian blur of f1, B ------------------
        if upto >= 3:
            # head special: output rows 18..21 with top-edge replicate
            # (clamped taps), plain [64, ...] layout.
            th = rowp.tile([64, 7, WP], F32R, name="bl_th", tag="cin", bufs=2)
            nc.sync.dma_start(out=th[:], in_=f1b[:, 18:25, :])
            vth = bigp.tile([64, 4, WP], F32R, name="vth", tag="blvt", bufs=2)
            for u in range(2):
                r0 = 18 + 2 * u
                psv = psp2.tile([64, 2, W], F32, name="blh_psv", tag="ps2")
                for dd in range(7):
                    m1_ = max(r0 + dd - 3, 18) - 18
                    m2_ = max(r0 + 1 + dd - 3, 18) - 18
                    if m1_ == m2_:
                        mv = th[:, m1_:m1_ + 1, X0:X0 + W].to_broadcast([64, 2, W])
                    else:
                        mv = th[:, m1_:m1_ + 2, X0:X0 + W]
                    nc.tensor.matmul(psv[:], w_bd[:, dd * 64:(dd + 1) * 64], mv,
                                     start=(dd == 0), stop=(dd == 6))
                nc.scalar.activation(vth[:, 2 * u:2 * u + 2, X0:X0 + W], psv[:],
                                     AF.Identity)
            nc.vector.tensor_copy(vth[:, :, 1:4],
                                  vth[:, :, X0:X0 + 1].to_broadcast([64, 4, 3]))
            nc.vector.tensor_copy(vth[:, :, WP - 4:WP - 1],
                                  vth[:, :, X0 + W - 1:X0 + W].to_broadcast([64, 4, 3]))
            fth = bigp.tile([64, 4, W], F32R, name="fth", tag="blft", bufs=2)
            bth = bigp.tile([2, 4, W], F32R, name="bth", tag="blbt", bufs=2)
            for u in range(2):
                psh = psp.tile([64, 2, W], F32, name="blh_psh", tag="ps")
                for j in range(7):
                    nc.tensor.matmul(psh[:], w_bd[:, j * 64:(j + 1) * 64],
                                     vth[:, 2 * u:2 * u + 2, 1 + j:1 + j + W],
                                     start=(j == 0), stop=(j == 6))
                nc.scalar.activation(fth[:, 2 * u:2 * u + 2, :], psh[:], AF.Identity)
                sqh = bigp.tile([64, 2, W], F32R, name="bl_sqh", tag="sqp", bufs=2)
                nc.scalar.activation(sqh[:], psh[:], AF.Square)
                psb = psp2.tile([2, 2, W], F32, name="blh_psb", tag="ps2")
                nc.tensor.matmul(psb[0:1, :, :], c_bd2[0:64, 0:1], sqh[:],
                                 start=True, stop=True)
                nc.scalar.activation(bth[0:1, 2 * u:2 * u + 2, :], psb[0:1, :, :],
                                     AF.Identity)
            nc.scalar.dma_start(out=f1sb[:, 18:22, X0:X0 + W], in_=fth[:])
            nc.scalar.dma_start(out=bb[0:1, 18:22, X0:X0 + W], in_=bth[0:1, :, :])

            # pair blocks: output rows 22..159
            for a in list(range(22, 158, BLK)) + [158]:
                nb = min(BLK, 160 - a)
                npair = nb // 2
                nslot = 2 * (npair - 1) + 7  # s = 2(t+j), t<=3, j<=npair-1
                pt = rowp.tile([128, 13, WP], F32R, name="bl_pt", tag="pair13", bufs=2)
                nc.sync.dma_start(
                    out=pt[:, 0:nslot, :],
                    in_=bass.AP(f1b, (a - 3) * WP,
                                [[WP, 2], [SLAB * WP, 64], [WP, nslot], [1, WP]]))
                vt = bigp.tile([128, 4, WP], F32R, name="bl_vt", tag="blvt", bufs=2)
                for u in range(0, npair, 2):
                    npr = min(2, npair - u)
                    psv = psp.tile([128, 2, W], F32, name="blp_psv", tag="ps")
                    for t in range(4):
                        s0 = 2 * (t + u)
                        nc.tensor.matmul(psv[:, 0:npr, :],
                                         w_vst[:, t * 128:(t + 1) * 128],
                                         pt[:, s0:s0 + 2 * npr - 1:2, X0:X0 + W],
                                         start=(t == 0), stop=(t == 3))
                    nc.scalar.activation(vt[:, u:u + npr, X0:X0 + W],
                                         psv[:, 0:npr, :], AF.Identity)
                nc.vector.tensor_copy(vt[:, 0:npair, 1:4],
                                      vt[:, 0:npair, X0:X0 + 1].to_broadcast([128, npair, 3]))
                nc.vector.tensor_copy(vt[:, 0:npair, WP - 4:WP - 1],
                                      vt[:, 0:npair, X0 + W - 1:X0 + W].to_broadcast([128, npair, 3]))
                ft = bigp.tile([128, 4, W], F32R, name="bl_ft", tag="blft", bufs=2)
                btl = bigp.tile([2, 4, W], F32R, name="bl_bt", tag="blbt", bufs=2)
                for u in range(0, npair, 2):
                    npr = min(2, npair - u)
                    psh = psp.tile([128, 2, W], F32, name="blp_psh", tag="ps")
                    for j in range(7):
                        nc.tensor.matmul(psh[:, 0:npr, :],
                                         w_bh128[:, j * 128:(j + 1) * 128],
                                         vt[:, u:u + npr, 1 + j:1 + j + W],
                                         start=(j == 0), stop=(j == 6))
                    nc.scalar.activation(ft[:, u:u + npr, :], psh[:, 0:npr, :],
                                         AF.Identity)
                    sqp = bigp.tile([128, 2, W], F32R, name="bl_sqp", tag="sqp", bufs=2)
                    nc.scalar.activation(sqp[:, 0:npr, :], psh[:, 0:npr, :], AF.Square)
                    psb = psp2.tile([2, 2, W], F32, name="blp_psb", tag="ps2")
                    nc.tensor.matmul(psb[:, 0:npr, :], c_bd2[:], sqp[:, 0:npr, :],
                                     start=True, stop=True)
                    nc.scalar.activation(btl[:, u:u + npr, :], psb[:, 0:npr, :],
                                         AF.Identity)
                nc.scalar.dma_start(out=f1sb[:, a:a + nb:2, X0:X0 + W],
                                    in_=ft[0:64, 0:npair, :])
                nc.scalar.dma_start(out=f1sb[:, a + 1:a + nb:2, X0:X0 + W],
                                    in_=ft[64:128, 0:npair, :])
                nc.scalar.dma_start(
                    out=bass.AP(bb, a * WP + X0, [[WP, 2], [2 * WP, npair], [1, W]]),
                    in_=btl[:, 0:npair, :])

        # ---------------- PASS 4: correlation -----------------------------
        if upto >= 4:
            cblocks = list(range(18, 147, BLK)) + [153]
            kstat = 0
            for a in cblocks:
                npair = 2 if a == 153 else 4
                nsf = 2 * npair - 1
                nsp = 2 * npair + 5  # slots 0 .. 2*(npair-2)+3+3+2
                f2t = rowp.tile([128, 7, WP], F32R, name="co_f2", tag="cin", bufs=2)
                nc.sync.dma_start(
                    out=f2t[:, 0:nsf, :],
                    in_=bass.AP(f2b, a * WP,
                                [[WP, 2], [SLAB * WP, 64], [WP, nsf], [1, WP]]))
                pt9 = rowp.tile([128, 13, WP], F32R, name="co_pt", tag="pair13", bufs=2)
                nc.sync.dma_start(
                    out=pt9[:, 0:nsp, :],
                    in_=bass.AP(f1sb, (a - 3) * WP,
                                [[WP, 2], [SLAB * WP, 64], [WP, nsp], [1, WP]]))
                bw = rowp.tile([56, 4, 528], F32R, name="co_bw", tag="cobw", bufs=2)
                nc.sync.dma_start(
                    out=bw[:, 0:npair, :],
                    in_=bass.AP(bb, (a - 3) * WP + X0 - 3,
                                [[1, 7], [WP, 8], [1, npair * 528]]))
                ct = bigp.tile([98, 4, W], F32R, name="co_ct", tag="coct", bufs=2)
                for u in range(0, npair, 2):
                    sqa = bigp.tile([128, 2, W], F32R, name="co_sqa", tag="sqp", bufs=2)
                    nc.scalar.activation(sqa[:], f2t[:, 2 * u:2 * u + 3:2, X0:X0 + W],
                                         AF.Square)
                    psa = psp2.tile([2, 2, W], F32, name="co_psa", tag="ps2")
                    nc.tensor.matmul(psa[:], c_bd2[:], sqa[:], start=True, stop=True)
                    at = bigp.tile([2, 2, W], F32R, name="co_at", tag="coat", bufs=2)
                    nc.scalar.activation(at[:], psa[:], AF.Identity)
                    psc = psp.tile([98, 2, W], F32, name="co_psc", tag="ps")
                    for i in range(7):
                        sh = i - 3
                        for j in range(7):
                            kk = i * 7 + j
                            s0 = 2 * u + sh + 3
                            prod = bigp.tile([128, 2, W], F32R, name="co_pr",
                                             tag="copr", bufs=3)
                            nc.vector.tensor_tensor(
                                prod[:], f2t[:, 2 * u:2 * u + 3:2, X0:X0 + W],
                                pt9[:, s0:s0 + 3:2, X0 + j - 3:X0 + j - 3 + W],
                                ALU.mult)
                            nc.tensor.matmul(psc[:], w_cm[:, kk * 98:(kk + 1) * 98],
                                             prod[:], start=(kk == 0), stop=False)
                    nc.tensor.matmul(psc[:], w_bsel56[:], bw[:, u:u + 2, 0:W],
                                     start=False, stop=False)
                    nc.tensor.matmul(psc[:], w_asel[:], at[:], start=False, stop=True)
                    if a + 2 * u + 3 <= STAT_HI:
                        cred = outp.tile([98, 1], F32, name="co_cred", tag="small",
                                         bufs=3)
                        nc.vector.tensor_reduce(
                            cred[:], psc[:].rearrange("p a x -> p (a x)"), AX.X,
                            ALU.add)
                        nc.vector.tensor_copy(st_c[:, kstat:kstat + 1], cred[:])
                        kstat += 1
                    nc.scalar.activation(ct[:, u:u + 2, :], psc[:], AF.Identity)
                nc.scalar.dma_start(
                    out=bass.AP(lcb, a * WP + X0,
                                [[SLAB * WP, 49], [2 * WP, npair], [1, W]]),
                    in_=ct[0:49, 0:npair, :])
                nc.scalar.dma_start(
                    out=bass.AP(lcb, (a + 1) * WP + X0,
                                [[SLAB * WP, 49], [2 * WP, npair], [1, W]]),
                    in_=ct[49:98, 0:npair, :])

        # ---------------- AR1 ---------------------------------------------
        if upto >= 5:
            fs = statp.tile([128, 1], F32, name="fs")
            fq = statp.tile([128, 1], F32, name="fq")
            nc.vector.tensor_reduce(fs[:], st_fc1_s[:], AX.X, ALU.add)
            nc.vector.tensor_reduce(fq[:], st_fc1_q[:], AX.X, ALU.add)
            credp = statp.tile([98, 1], F32, name="credp")
            nc.vector.tensor_reduce(credp[:], st_c[:], AX.X, ALU.add)
            # C partial: flatten [98,1] -> [1,98] via DRAM, reduce to scalar
            nc.sync.dma_start(out=scr[0:1, 0:98], in_=credp[:])
            ctot_r = statp.tile([1, 98], F32, name="ctot_r")
            nc.sync.dma_start(out=ctot_r[:], in_=scr[0:1, 0:98])
            ctot = statp.tile([1, 1], F32, name="ctot")
            nc.vector.tensor_reduce(ctot[:], ctot_r[:], AX.X, ALU.add)
            # payload: [0:128) sum, [128:256) sumsq, 256 = lc total
            nc.sync.dma_start(out=ar1i[0:1, 0:128], in_=fs[:])
            nc.sync.dma_start(out=ar1i[0:1, 128:256], in_=fq[:])
            nc.sync.dma_start(out=ar1i[0:1, 256:257], in_=ctot[:])
            if collectives:
                nc.gpsimd.collective_compute("AllReduce", ALU.add, replica_groups=GROUPS,
                                             ins=[ar1i[:, :]], outs=[ar1o[:, :]])
            else:
                nc.sync.dma_start(out=ar1o[:, :], in_=ar1i[:, :])
            fc1s = statp.tile([128, 1], F32, name="fc1s")
            fc1q = statp.tile([128, 1], F32, name="fc1q")
            nc.sync.dma_start(out=fc1s[:], in_=ar1o[0:1, 0:128])
            nc.sync.dma_start(out=fc1q[:], in_=ar1o[0:1, 128:256])
            sc = statp.tile([1, 1], F32, name="sc")
            nc.sync.dma_start(out=sc[:], in_=ar1o[0:1, 256:257])

            def norm_params(ssum, sq, gam, bet, n, c, nm):
                mu = statp.tile([c, 1], F32, name=f"mu_{nm}")
                nc.vector.tensor_scalar_mul(mu[:], ssum[:], 1.0 / n)
                var = statp.tile([c, 1], F32, name=f"var_{nm}")
                nc.vector.tensor_scalar_mul(var[:], sq[:], 1.0 / n)
                mu2 = statp.tile([c, 1], F32, name=f"mu2_{nm}")
                nc.vector.tensor_tensor(mu2[:], mu[:], mu[:], ALU.mult)
                nc.vector.tensor_tensor(var[:], var[:], mu2[:], ALU.subtract)
                nc.vector.tensor_scalar_add(var[:], var[:], EPS_IN)
                sd = statp.tile([c, 1], F32, name=f"sd_{nm}")
                nc.scalar.activation(sd[:], var[:], AF.Sqrt)
                rstd = statp.tile([c, 1], F32, name=f"rstd_{nm}")
                nc.vector.reciprocal(rstd[:], sd[:])
                s_ = statp.tile([c, 1], F32, name=f"s_{nm}")
                nc.vector.tensor_tensor(s_[:], gam[:], rstd[:], ALU.mult)
                b_ = statp.tile([c, 1], F32, name=f"b_{nm}")
                nc.vector.tensor_tensor(b_[:], mu[:], s_[:], ALU.mult)
                nc.vector.tensor_tensor(b_[:], bet[:], b_[:], ALU.subtract)
                return s_[:, 0:1], b_[:, 0:1]

            fc1_sn, fc1_bn = norm_params(fc1s, fc1q, c_fc1_g, c_fc1_be, 65536.0, 128, "fc1")

            # s = lcT / (49*H*W) + eps ; rs = 1/s ; scale e1 lc-channel weights
            sval = statp.tile([1, 1], F32, name="sval")
            nc.vector.tensor_scalar(sval[:], sc[0:1, 0:1], 1.0 / (49.0 * H * W), None,
                                    ALU.mult)
            nc.vector.tensor_scalar_add(sval[:], sval[:], EPS_CORR)
            rsv = statp.tile([1, 1], F32, name="rsv")
            nc.vector.reciprocal(rsv[:], sval[:])
            nc.sync.dma_start(out=scr[0:1, 100:101], in_=rsv[:])
            rb113 = statp.tile([113, 1], F32, name="rb113")
            nc.sync.dma_start(out=rb113[:],
                              in_=bass.AP(scr, 100, [[0, 113], [1, 1]]))
            fac = statp.tile([113, 1], F32, name="fac")
            nc.vector.tensor_tensor(fac[:], c_m1[:], rb113[:], ALU.mult)
            nc.vector.tensor_tensor(fac[:], fac[:], c_m0[:], ALU.add)
            nc.vector.tensor_scalar(w_e1[:], w_e1[:], fac[:, 0:1], None, ALU.mult)

        # ---------------- PASS 5: fc2 -------------------------------------
        if upto >= 6:
            conv_pass([(t1b, 128)], featb, w_fc2, 128, 64, 1, ROWS_FEAT, "fc2",
                      bias=c_fc2_b[:, 0:1], norm=(fc1_sn, fc1_bn))

        # ---------------- PASS 6: e1 --------------------------------------
        if upto >= 7:
            conv_pass([(featb, 64), (lcb, 49)], e1b, w_e1, 113, 64, 1, ROWS_E1,
                      "e1", stats=(st_e1_s, st_e1_q))

        # ---- AR2 / e2 / AR3 / e3 / AR4 / head ---------------------------
        if upto >= 8:
            def stats_ar(st_s, st_q, c, ari, aro, nm):
                s1 = statp.tile([c, 1], F32, name=f"ss_{nm}")
                q1 = statp.tile([c, 1], F32, name=f"qq_{nm}")
                nc.vector.tensor_reduce(s1[:], st_s[:], AX.X, ALU.add)
                nc.vector.tensor_reduce(q1[:], st_q[:], AX.X, ALU.add)
                nc.sync.dma_start(out=ari[0:1, 0:c], in_=s1[:])
                nc.sync.dma_start(out=ari[0:1, c:2 * c], in_=q1[:])
                if collectives:
                    nc.gpsimd.collective_compute("AllReduce", ALU.add,
                                                 replica_groups=GROUPS,
                                                 ins=[ari[:, :]], outs=[aro[:, :]])
                else:
                    nc.sync.dma_start(out=aro[:, :], in_=ari[:, :])
                s2 = statp.tile([c, 1], F32, name=f"ss2_{nm}")
                q2 = statp.tile([c, 1], F32, name=f"qq2_{nm}")
                nc.sync.dma_start(out=s2[:], in_=aro[0:1, 0:c])
                nc.sync.dma_start(out=q2[:], in_=aro[0:1, c:2 * c])
                return s2, q2

            e1s2, e1q2 = stats_ar(st_e1_s, st_e1_q, 64, ar2i, ar2o, "e1")
            e1_sn, e1_bn = norm_params(e1s2, e1q2, c_e1_g, c_e1_be, 65536.0, 64, "e1")

            conv_pass([(e1b, 64)], e2b, w_e2, 64, 32, 2, ROWS_E2, "e2",
                      stats=(st_e2_s, st_e2_q), norm=(e1_sn, e1_bn))
            e2s2, e2q2 = stats_ar(st_e2_s, st_e2_q, 32, ar3i, ar3o, "e2")
            e2_sn, e2_bn = norm_params(e2s2, e2q2, c_e2_g, c_e2_be, 65536.0, 32, "e2")

            conv_pass([(e2b, 32)], e3b, w_e3, 32, 16, 4, ROWS_E3, "e3",
                      stats=(st_e3_s, st_e3_q), norm=(e2_sn, e2_bn))
            e3s2, e3q2 = stats_ar(st_e3_s, st_e3_q, 16, ar4i, ar4o, "e3")
            e3_sn, e3_bn = norm_params(e3s2, e3q2, c_e3_g, c_e3_be, 65536.0, 16, "e3")

            conv_pass([(e3b, 16)], headbuf, w_head, 16, 2, 1, ROWS_HEAD, "head",
                      bias=c_head_b[:, 0:1], norm=(e3_sn, e3_bn))

        # ---------------- PASS 10: disp blur ------------------------------
        if upto >= 9:
            blocks = [
                (w_bmv0, 118, 18, 59, 18, 56, 112),
                (w_bmv1, 124, 71, 62, 74, 56, 112),
                (w_bmv2, 44, 127, 22, 130, 16, 32),
            ]
            for wm, kp, rin0, nin, ro0, nout, mp in blocks:
                ht = bigp.tile([kp, WP], F32R, name="dh_in", tag="bigA")
                nc.sync.dma_start(
                    out=ht[:],
                    in_=headbuf[:, rin0:rin0 + nin, :].rearrange("c r x -> r c x"))
                psv = psp.tile([mp, WP], F32, name="dps_v", tag="ps")
                nc.tensor.matmul(psv[:], wm[:], ht[:], start=True, stop=True)
                vt = bigp.tile([mp, WP], F32R, name="dvt", tag="bigB")
                nc.scalar.activation(vt[:], psv[:], AF.Identity)
                nc.vector.tensor_copy(vt[:, 1:4], vt[:, X0:X0 + 1].to_broadcast([mp, 3]))
                nc.vector.tensor_copy(vt[:, WP - 4:WP - 1],
                                      vt[:, X0 + W - 1:X0 + W].to_broadcast([mp, 3]))
                psh = psp.tile([mp, W], F32, name="dps_h", tag="ps")
                for j in range(7):
                    nc.tensor.matmul(psh[:], w_bh112[0:mp, j * 112:j * 112 + mp],
                                     vt[:, 1 + j:1 + j + W], start=(j == 0), stop=(j == 6))
                ot = outp.tile([mp, W], F32, name="do", tag="small")
                nc.scalar.activation(ot[:], psh[:], AF.Identity)
                nc.sync.dma_start(
                    out=out[:, ro0 - R0:ro0 - R0 + nout, :].rearrange("c r x -> r c x"),
                    in_=ot[:])

    _fix_waits(nc)
    return nc


# ---------------------------------------------------------------- entry
_NC = None


def kernel(**inputs):
    global _NC
    if _NC is None:
        _NC = build_module()
    in_maps = [_prep_core_inputs(inputs, c) for c in range(8)]
    res = run_bass_kernel_spmd(_NC, in_maps, list(range(8)))
    disp = np.zeros((B, 2, H, W), np.float32)
    for c in range(8):
        b, h = c // 2, c % 2
        o = res.results[c]["out"]
        if h == 0:
            disp[b, :, 0:HALF, :] = o
        else:
            disp[b, :, HALF:H, :] = o[:, ::-1, :]
    return disp
